# Initial kernel scaffold
#
"""Trainium2 Bass kernel for CannyExtractor (NMS-suppressed canny magnitude).

Contract: kernel(x) takes the FULL input x [16,3,512,512] f32 and returns the
FULL output [16,3,512,512] f32. Internally shards the batch over 8 NeuronCores
(2 images per core), runs one SPMD Bass program, and reassembles.

Pipeline per image (all fp32):
  gray -> vertical composite convs on PE (banded matmuls, block-diagonal with
  a stacked corner matmul for the 3-row inter-block halos) -> horizontal convs
  on DVE (Horner chains of scalar_tensor_tensor) -> squared magnitudes ->
  NMS axis selection via squared-gradient comparisons (exactly equivalent to
  the reference's round(atan2)/45 quantization) -> suppression -> clip.
Row +/-1 shifted planes for NMS come from exact fp32 permutation matmuls on
PE, with cross-block corner rows patched by tiny SBUF-to-SBUF DMAs.
"""
import sys
import numpy as np

sys.path.insert(0, "/opt/trn_rl_repo")

H = W = 512
NT = 4            # 128-row blocks per image
P = 128
PAD = 2
WP = W + 2 * PAD  # padded plane width
NI = 2            # images per core
NCORES = 8

GRAY = np.array([0.299, 0.587, 0.114], np.float32)
T2 = np.float32((np.sqrt(2.0) - 1.0) ** 2)   # tan^2(22.5 deg)
EPS = np.float32(1e-6)


def _gauss5():
    ax = np.arange(5, dtype=np.float32) - 2.0
    g1 = np.exp(-0.5 * ax * ax).astype(np.float32)
    return (g1 / g1.sum()).astype(np.float32)


def _vert_matrix(kind):
    """512x512 M[o,i]: vertical composite (3-tap sobel part o replicate-pad o
    gaussian o reflect-pad), float64."""
    g1 = _gauss5()
    I = np.eye(H, dtype=np.float64)
    X = np.pad(I, ((2, 2), (0, 0)), mode="reflect")
    B = np.zeros((H, H))
    for k in range(5):
        B += g1[k] * X[k:k + H]
    Y = np.pad(B, ((1, 1), (0, 0)), mode="edge")
    taps = [1.0, 2.0, 1.0] if kind == "smooth" else [-1.0, 0.0, 1.0]
    M = np.zeros((H, H))
    for k in range(3):
        if taps[k] != 0.0:
            M += taps[k] * Y[k:k + H]
    return M


def _build_consts():
    Ms = (_vert_matrix("smooth") * float(GRAY[2])).astype(np.float32)
    Md = (_vert_matrix("diff") * float(GRAY[2])).astype(np.float32)
    # main block-diagonal bands: vs[k, t, m] = M[128t+m, 128t+k]
    vs = np.zeros((P, NT, P), np.float32)
    vd = np.zeros((P, NT, P), np.float32)
    for t in range(NT):
        vs[:, t, :] = Ms[128 * t:128 * (t + 1), 128 * t:128 * (t + 1)].T
        vd[:, t, :] = Md[128 * t:128 * (t + 1), 128 * t:128 * (t + 1)].T
    # corner matmul: stacked strips (3 boundaries x 12 in-rows) -> 18 out-rows
    vcor = np.zeros((36, 2, 18), np.float32)
    for b in range(3):
        in_rows = [128 * b + 122 + k for k in range(12)]
        out_rows = [128 * b + 125, 128 * b + 126, 128 * b + 127,
                    128 * (b + 1), 128 * (b + 1) + 1, 128 * (b + 1) + 2]
        for k, ir in enumerate(in_rows):
            for m, orr in enumerate(out_rows):
                vcor[12 * b + k, 0, 6 * b + m] = Ms[orr, ir]
                vcor[12 * b + k, 1, 6 * b + m] = Md[orr, ir]
    # shift matrices: sup[k,m]=1 iff k=m+1 (U[m]=s[m+1]); sdn[k,m]=1 iff k=m-1
    shm = np.zeros((P, 2, P), np.float32)
    for m in range(P - 1):
        shm[m + 1, 0, m] = 1.0
    for m in range(1, P):
        shm[m - 1, 1, m] = 1.0
    return {"vs": vs, "vd": vd, "vcor": vcor, "shm": shm}


_CACHE = {}


def _emit_image(nc, tc, pools, tens, img):
    """Generator: yields between pipeline stages so the caller can interleave
    the two images' stages for cross-image engine overlap. All elementwise
    ops are emitted per half-plane (2 of the 4 row-blocks) for finer
    dependency granularity."""
    import concourse.mybir as mybir
    AL = mybir.AluOpType
    AF = mybir.ActivationFunctionType
    F32 = mybir.dt.float32
    U8 = mybir.dt.uint8

    pwork, pmask, psmall, ppsum = pools
    xdram, ydram, c_vs, c_vd, c_vcor, c_shm, zeros, epsb = tens

    g1 = _gauss5()
    C0, C1, C2 = float(g1[2]), float(g1[1]), float(g1[0])
    R01 = float(np.float32(GRAY[0] / GRAY[1]))
    R12 = float(np.float32(GRAY[1] / GRAY[2]))

    INT = slice(PAD, PAD + W)      # interior columns of padded planes
    NW = slice(0, W)               # column range for unpadded use
    BS = tuple(slice(i, i + 1) for i in range(NT))  # per-block ranges

    import os
    UPTO = int(os.environ.get("KSTAGES", "99"))
    stage = [0]

    def gate():
        stage[0] += 1
        return stage[0] >= UPTO

    def wt():
        return pwork.tile([P, NT, WP], F32, tag="w", name="w")

    # ---- load input channels (per half) ----
    xc = []
    for c in range(3):
        t = wt()
        for hs in BS:
            nc.sync.dma_start(
                t[:, hs, NW],
                xdram[img, c].rearrange("(t p) w -> p t w", p=P)[:, hs, :])
        xc.append(t)

    # ---- grayscale (final x0.114 folded into the vertical matrices) ----
    gtmp = wt()
    g = wt()
    for hs in BS:
        nc.vector.scalar_tensor_tensor(gtmp[:, hs, NW], xc[0][:, hs, NW], R01,
                                       xc[1][:, hs, NW], AL.mult, AL.add)
        nc.vector.scalar_tensor_tensor(g[:, hs, NW], gtmp[:, hs, NW], R12,
                                       xc[2][:, hs, NW], AL.mult, AL.add)
    yield
    if gate():
        return

    # ---- vertical composite convs on PE ----
    u1 = wt()
    u2 = wt()
    for t in range(NT):
        for (cm, u) in ((c_vs, u1), (c_vd, u2)):
            psb = ppsum.tile([P, W], F32, tag="ps", name="ps")
            nc.tensor.matmul(psb[:], cm[:, t, :], g[:, t, NW], start=True, stop=True)
            nc.scalar.copy(u[:, t, INT], psb[:])
    # corner strips: stack 12 in-rows per boundary into one [36, W] tile
    cs = psmall.tile([36, W], F32, tag="cs", name="cs")
    for b in range(3):
        nc.sync.dma_start(cs[12 * b:12 * b + 6, :], g[122:128, b, NW])
        nc.sync.dma_start(cs[12 * b + 6:12 * b + 12, :], g[0:6, b + 1, NW])
    for ci, u in ((0, u1), (1, u2)):
        cps = ppsum.tile([18, W], F32, tag="ps", name="ps")
        nc.tensor.matmul(cps[:], c_vcor[:, ci, :], cs[:], start=True, stop=True)
        co = psmall.tile([18, W], F32, tag="co", name="co")
        nc.scalar.copy(co[:], cps[:])
        for b in range(3):
            nc.sync.dma_start(u[125:128, b, INT], co[6 * b:6 * b + 3, :])
            nc.sync.dma_start(u[0:3, b + 1, INT], co[6 * b + 3:6 * b + 6, :])
    # reflect guard columns for the horizontal gaussian
    for u in (u1, u2):
        nc.scalar.copy(u[:, :, 1:2], u[:, :, 3:4])
        nc.scalar.copy(u[:, :, 0:1], u[:, :, 4:5])
        nc.scalar.copy(u[:, :, WP - 2:WP - 1], u[:, :, WP - 4:WP - 3])
        nc.scalar.copy(u[:, :, WP - 1:WP], u[:, :, WP - 5:WP - 4])
    yield
    if gate():
        return

    # ---- horizontal gaussian (5-tap, Horner; final xC0 folded into Square) ----
    bl = []
    for u in (u1, u2):
        a1 = wt()
        a2 = wt()
        q1 = wt()
        b_ = wt()
        for hs in BS:
            nc.gpsimd.tensor_tensor(a1[:, hs, NW], u[:, hs, 1:1 + W],
                                    u[:, hs, 3:3 + W], AL.add)
            nc.gpsimd.tensor_tensor(a2[:, hs, NW], u[:, hs, 0:W],
                                    u[:, hs, 4:4 + W], AL.add)
            nc.vector.scalar_tensor_tensor(q1[:, hs, NW], a2[:, hs, NW], C2 / C1,
                                           a1[:, hs, NW], AL.mult, AL.add)
            nc.vector.scalar_tensor_tensor(b_[:, hs, INT], q1[:, hs, NW], C1 / C0,
                                           u[:, hs, INT], AL.mult, AL.add)
        # replicate guard columns for the 3-tap stage
        nc.scalar.copy(b_[:, :, 1:2], b_[:, :, 2:3])
        nc.scalar.copy(b_[:, :, WP - 2:WP - 1], b_[:, :, WP - 3:WP - 2])
        bl.append(b_)
    b1, b2 = bl
    yield
    if gate():
        return

    # ---- gradients (scaled by 1/C0), squares, s = gx^2+gy^2 ----
    gx = wt()
    ay = wt()
    gy = wt()
    sqx = wt()
    sqy = wt()
    s = wt()
    pxy = wt()
    md1 = pmask.tile([P, NT, W], U8, tag="m", name="m")
    ch = pmask.tile([P, NT, W], U8, tag="m", name="m")
    cv = pmask.tile([P, NT, W], U8, tag="m", name="m")
    for hs in BS:
        nc.gpsimd.tensor_tensor(gx[:, hs, NW], b1[:, hs, 3:3 + W],
                                b1[:, hs, 1:1 + W], AL.subtract)
        nc.gpsimd.tensor_tensor(ay[:, hs, NW], b2[:, hs, 1:1 + W],
                                b2[:, hs, 3:3 + W], AL.add)
        nc.vector.scalar_tensor_tensor(gy[:, hs, NW], b2[:, hs, INT], 2.0,
                                       ay[:, hs, NW], AL.mult, AL.add)
        nc.scalar.activation(sqx[:, hs, NW], gx[:, hs, NW], AF.Square, 0.0, C0)
        nc.scalar.activation(sqy[:, hs, NW], gy[:, hs, NW], AF.Square, 0.0, C0)
        nc.gpsimd.tensor_tensor(s[:, hs, INT], sqx[:, hs, NW], sqy[:, hs, NW],
                                AL.add)
        nc.gpsimd.tensor_tensor(pxy[:, hs, NW], gx[:, hs, NW], gy[:, hs, NW],
                                AL.mult)
        nc.vector.tensor_scalar(md1[:, hs, :], pxy[:, hs, NW], 0.0, None, AL.is_gt)
        nc.vector.scalar_tensor_tensor(ch[:, hs, :], sqx[:, hs, NW], float(T2),
                                       sqy[:, hs, NW], AL.mult, AL.is_ge)
        nc.vector.scalar_tensor_tensor(cv[:, hs, :], sqy[:, hs, NW], float(T2),
                                       sqx[:, hs, NW], AL.mult, AL.is_gt)
    nc.gpsimd.memset(s[:, :, 0:PAD], 0.0)
    nc.gpsimd.memset(s[:, :, WP - PAD:WP], 0.0)
    yield
    if gate():
        return

    # ---- row-shifted planes U[r]=s[r+1], D[r]=s[r-1] via exact PE permutation ----
    Upl = wt()
    Dpl = wt()
    for t in range(NT):
        for (ci, pl) in ((0, Upl), (1, Dpl)):
            psb = ppsum.tile([P, W], F32, tag="ps", name="ps")
            nc.tensor.matmul(psb[:], c_shm[:, ci, :], s[:, t, INT],
                             start=True, stop=True)
            nc.scalar.copy(pl[:, t, INT], psb[:])
    for pl in (Upl, Dpl):
        nc.gpsimd.memset(pl[:, :, 0:PAD], 0.0)
        nc.gpsimd.memset(pl[:, :, WP - PAD:WP], 0.0)
    for t in range(NT - 1):
        nc.sync.dma_start(Upl[127:128, t, INT], s[0:1, t + 1, INT])
    nc.sync.dma_start(Upl[127:128, NT - 1, INT], zeros[0:1, 0, :])
    for t in range(1, NT):
        nc.sync.dma_start(Dpl[0:1, t, INT], s[127:128, t - 1, INT])
    nc.gpsimd.memset(Dpl[0:1, 0, INT], 0.0)
    yield
    if gate():
        return

    # ---- neighbor maxes, axis selection, output ----
    mh = wt()
    mv = wt()
    md1m = wt()
    sel = wt()
    mag = wt()
    magc = wt()
    keep = wt()
    out_ = wt()
    for hs in BS:
        nc.vector.tensor_tensor(mh[:, hs, NW], s[:, hs, 1:1 + W],
                                s[:, hs, 3:3 + W], AL.max)
        nc.vector.tensor_tensor(mv[:, hs, NW], Upl[:, hs, INT], Dpl[:, hs, INT],
                                AL.max)
        nc.vector.tensor_tensor(md1m[:, hs, NW], Upl[:, hs, 3:3 + W],
                                Dpl[:, hs, 1:1 + W], AL.max)
        nc.vector.tensor_tensor(sel[:, hs, NW], Upl[:, hs, 1:1 + W],
                                Dpl[:, hs, 3:3 + W], AL.max)
        nc.vector.copy_predicated(sel[:, hs, NW], md1[:, hs, :], md1m[:, hs, NW])
        nc.vector.copy_predicated(sel[:, hs, NW], cv[:, hs, :], mv[:, hs, NW])
        nc.vector.copy_predicated(sel[:, hs, NW], ch[:, hs, :], mh[:, hs, NW])
        nc.scalar.activation(mag[:, hs, NW], s[:, hs, INT], AF.Sqrt, epsb[:], 1.0)
        nc.vector.tensor_scalar_min(magc[:, hs, NW], mag[:, hs, NW], 1.0)
        nc.vector.tensor_tensor(keep[:, hs, NW], s[:, hs, INT], sel[:, hs, NW],
                                AL.is_gt)
        nc.gpsimd.tensor_tensor(out_[:, hs, NW], magc[:, hs, NW],
                                keep[:, hs, NW], AL.mult)
        for c in range(3):
            nc.sync.dma_start(
                ydram[img, c].rearrange("(t p) w -> p t w", p=P)[:, hs, :],
                out_[:, hs, NW])
    yield
    if gate():
        return


def _build():
    import concourse.bacc as bacc
    import concourse.mybir as mybir
    from concourse import tile
    F32 = mybir.dt.float32

    nc = bacc.Bacc("TRN2", target_bir_lowering=False, debug=False,
                   num_devices=NCORES)
    xdram = nc.declare_dram_parameter("xc", [NI, 3, H, W], F32, isOutput=False)
    c_vs_d = nc.declare_dram_parameter("vs", [P, NT, P], F32, isOutput=False)
    c_vd_d = nc.declare_dram_parameter("vd", [P, NT, P], F32, isOutput=False)
    c_vcor_d = nc.declare_dram_parameter("vcor", [36, 2, 18], F32, isOutput=False)
    c_shm_d = nc.declare_dram_parameter("shm", [P, 2, P], F32, isOutput=False)
    ydram = nc.declare_dram_parameter("y", [NI, 3, H, W], F32, isOutput=True)

    with tile.TileContext(nc) as tc:
        with tc.tile_pool(name="pconst", bufs=1) as pconst, \
             tc.tile_pool(name="pwork", bufs=21) as pwork, \
             tc.tile_pool(name="pmask", bufs=6) as pmask, \
             tc.tile_pool(name="psmall", bufs=2) as psmall, \
             tc.tile_pool(name="ppsum", bufs=6, space="PSUM") as ppsum:
            c_vs = pconst.tile([P, NT, P], F32, tag="cvs")
            nc.sync.dma_start(c_vs[:], c_vs_d[:])
            c_vd = pconst.tile([P, NT, P], F32, tag="cvd")
            nc.sync.dma_start(c_vd[:], c_vd_d[:])
            c_vcor = pconst.tile([36, 2, 18], F32, tag="cvcor")
            nc.sync.dma_start(c_vcor[:], c_vcor_d[:])
            c_shm = pconst.tile([P, 2, P], F32, tag="cshm")
            nc.sync.dma_start(c_shm[:], c_shm_d[:])
            zeros = pconst.tile([P, NT, W], F32, tag="zeros")
            nc.gpsimd.memset(zeros[:], 0.0)
            epsb = pconst.tile([P, 1], F32, tag="epsb")
            nc.gpsimd.memset(epsb[:], float(EPS))

            pools = (pwork, pmask, psmall, ppsum)
            tens = (xdram, ydram, c_vs, c_vd, c_vcor, c_shm, zeros, epsb)
            import os
            nrep = int(os.environ.get("KREPEAT", "1"))
            for rep in range(nrep):
                gens = [_emit_image(nc, tc, pools, tens, img) for img in range(NI)]
                done = [False] * NI
                while not all(done):
                    for i, gi in enumerate(gens):
                        if not done[i]:
                            try:
                                next(gi)
                            except StopIteration:
                                done[i] = True

    nc.compile()
    return nc


def _get_nc():
    if "nc" not in _CACHE:
        _CACHE["nc"] = _build()
        _CACHE["consts"] = _build_consts()
    return _CACHE["nc"], _CACHE["consts"]


def kernel(x):
    from concourse.bass_utils import run_bass_kernel_spmd
    x = np.ascontiguousarray(np.asarray(x), dtype=np.float32)
    assert x.shape == (16, 3, H, W), x.shape
    nc, consts = _get_nc()
    in_maps = []
    for c in range(NCORES):
        m = {"xc": x[NI * c:NI * (c + 1)]}
        m.update(consts)
        in_maps.append(m)
    res = run_bass_kernel_spmd(nc, in_maps, list(range(NCORES)))
    y = np.concatenate([res.results[c]["y"] for c in range(NCORES)], axis=0)
    return y.astype(np.float32)


if __name__ == "__main__":
    import golden
    rng = np.random.default_rng(0)
    x = rng.random((16, 3, H, W), dtype=np.float32)
    y = kernel(x)
    ref = golden.reference_np(x)
    d = y - ref
    print("L2 rel:", np.linalg.norm(d) / np.linalg.norm(ref))
    print("absmax:", np.abs(d).max(), " bigpix:", (np.abs(d) > 1e-3).sum())



# revision 3
# speedup vs baseline: 1.0067x; 1.0067x over previous
"""Trainium2 Bass kernel for CannyExtractor (NMS-suppressed canny magnitude).

Contract: kernel(x) takes the FULL input x [16,3,512,512] f32 and returns the
FULL output [16,3,512,512] f32. Internally shards the batch over 8 NeuronCores
(2 images per core), runs one SPMD Bass program, and reassembles.

Pipeline per image (all fp32):
  gray -> vertical composite convs on PE (banded matmuls, block-diagonal with
  a stacked corner matmul for the 3-row inter-block halos) -> horizontal convs
  on DVE (Horner chains of scalar_tensor_tensor) -> squared magnitudes ->
  NMS axis selection via squared-gradient comparisons (exactly equivalent to
  the reference's round(atan2)/45 quantization) -> suppression -> clip.
Row +/-1 shifted planes for NMS come from exact fp32 permutation matmuls on
PE, with cross-block corner rows patched by tiny SBUF-to-SBUF DMAs.
"""
import sys
import numpy as np

sys.path.insert(0, "/opt/trn_rl_repo")

H = W = 512
NT = 4            # 128-row blocks per image
P = 128
PAD = 2
WP = W + 2 * PAD  # padded plane width
NI = 2            # images per core
NCORES = 8

GRAY = np.array([0.299, 0.587, 0.114], np.float32)
T2 = np.float32((np.sqrt(2.0) - 1.0) ** 2)   # tan^2(22.5 deg)
EPS = np.float32(1e-6)


def _gauss5():
    ax = np.arange(5, dtype=np.float32) - 2.0
    g1 = np.exp(-0.5 * ax * ax).astype(np.float32)
    return (g1 / g1.sum()).astype(np.float32)


def _vert_matrix(kind):
    """512x512 M[o,i]: vertical composite (3-tap sobel part o replicate-pad o
    gaussian o reflect-pad), float64."""
    g1 = _gauss5()
    I = np.eye(H, dtype=np.float64)
    X = np.pad(I, ((2, 2), (0, 0)), mode="reflect")
    B = np.zeros((H, H))
    for k in range(5):
        B += g1[k] * X[k:k + H]
    Y = np.pad(B, ((1, 1), (0, 0)), mode="edge")
    taps = [1.0, 2.0, 1.0] if kind == "smooth" else [-1.0, 0.0, 1.0]
    M = np.zeros((H, H))
    for k in range(3):
        if taps[k] != 0.0:
            M += taps[k] * Y[k:k + H]
    return M


def _build_consts():
    Ms = (_vert_matrix("smooth") * float(GRAY[2])).astype(np.float32)
    Md = (_vert_matrix("diff") * float(GRAY[2])).astype(np.float32)
    # main block-diagonal bands: vs[k, t, m] = M[128t+m, 128t+k]
    vs = np.zeros((P, NT, P), np.float32)
    vd = np.zeros((P, NT, P), np.float32)
    for t in range(NT):
        vs[:, t, :] = Ms[128 * t:128 * (t + 1), 128 * t:128 * (t + 1)].T
        vd[:, t, :] = Md[128 * t:128 * (t + 1), 128 * t:128 * (t + 1)].T
    # corner matmul: stacked strips (3 boundaries x 12 in-rows) -> 18 out-rows
    vcor = np.zeros((36, 2, 18), np.float32)
    for b in range(3):
        in_rows = [128 * b + 122 + k for k in range(12)]
        out_rows = [128 * b + 125, 128 * b + 126, 128 * b + 127,
                    128 * (b + 1), 128 * (b + 1) + 1, 128 * (b + 1) + 2]
        for k, ir in enumerate(in_rows):
            for m, orr in enumerate(out_rows):
                vcor[12 * b + k, 0, 6 * b + m] = Ms[orr, ir]
                vcor[12 * b + k, 1, 6 * b + m] = Md[orr, ir]
    # shift matrices: sup[k,m]=1 iff k=m+1 (U[m]=s[m+1]); sdn[k,m]=1 iff k=m-1
    shm = np.zeros((P, 2, P), np.float32)
    for m in range(P - 1):
        shm[m + 1, 0, m] = 1.0
    for m in range(1, P):
        shm[m - 1, 1, m] = 1.0
    return {"vs": vs, "vd": vd, "vcor": vcor, "shm": shm}


_CACHE = {}


def _emit_image(nc, tc, pools, tens, img):
    """Generator: yields between pipeline stages so the caller can interleave
    the two images' stages for cross-image engine overlap. All elementwise
    ops are emitted per half-plane (2 of the 4 row-blocks) for finer
    dependency granularity."""
    import concourse.mybir as mybir
    AL = mybir.AluOpType
    AF = mybir.ActivationFunctionType
    F32 = mybir.dt.float32
    U8 = mybir.dt.uint8

    pwork, pmask, psmall, ppsum = pools
    xdram, ydram, c_vs, c_vd, c_vcor, c_shm, zeros, epsb = tens

    g1 = _gauss5()
    C0, C1, C2 = float(g1[2]), float(g1[1]), float(g1[0])
    R01 = float(np.float32(GRAY[0] / GRAY[1]))
    R12 = float(np.float32(GRAY[1] / GRAY[2]))

    INT = slice(PAD, PAD + W)      # interior columns of padded planes
    NW = slice(0, W)               # column range for unpadded use
    BS = tuple(slice(i, i + 1) for i in range(NT))  # per-block ranges

    import os
    UPTO = int(os.environ.get("KSTAGES", "99"))
    stage = [0]

    def gate():
        stage[0] += 1
        return stage[0] >= UPTO

    def wt():
        return pwork.tile([P, NT, WP], F32, tag="w", name="w")

    # ---- load input channels (per half) ----
    xc = []
    for c in range(3):
        t = wt()
        for hs in BS:
            nc.sync.dma_start(
                t[:, hs, NW],
                xdram[img, c].rearrange("(t p) w -> p t w", p=P)[:, hs, :])
        xc.append(t)

    # ---- grayscale (final x0.114 folded into the vertical matrices) ----
    gtmp = wt()
    g = wt()
    for hs in BS:
        nc.vector.scalar_tensor_tensor(gtmp[:, hs, NW], xc[0][:, hs, NW], R01,
                                       xc[1][:, hs, NW], AL.mult, AL.add)
        nc.vector.scalar_tensor_tensor(g[:, hs, NW], gtmp[:, hs, NW], R12,
                                       xc[2][:, hs, NW], AL.mult, AL.add)
    yield
    if gate():
        return

    # ---- vertical composite convs on PE ----
    u1 = wt()
    u2 = wt()
    for t in range(NT):
        for (cm, u) in ((c_vs, u1), (c_vd, u2)):
            psb = ppsum.tile([P, W], F32, tag="ps", name="ps")
            nc.tensor.matmul(psb[:], cm[:, t, :], g[:, t, NW], start=True, stop=True)
            nc.scalar.copy(u[:, t, INT], psb[:])
    # corner strips: stack 12 in-rows per boundary into one [36, W] tile
    cs = psmall.tile([36, W], F32, tag="cs", name="cs")
    for b in range(3):
        nc.sync.dma_start(cs[12 * b:12 * b + 6, :], g[122:128, b, NW])
        nc.sync.dma_start(cs[12 * b + 6:12 * b + 12, :], g[0:6, b + 1, NW])
    for ci, u in ((0, u1), (1, u2)):
        cps = ppsum.tile([18, W], F32, tag="ps", name="ps")
        nc.tensor.matmul(cps[:], c_vcor[:, ci, :], cs[:], start=True, stop=True)
        co = psmall.tile([18, W], F32, tag="co", name="co")
        nc.scalar.copy(co[:], cps[:])
        for b in range(3):
            nc.sync.dma_start(u[125:128, b, INT], co[6 * b:6 * b + 3, :])
            nc.sync.dma_start(u[0:3, b + 1, INT], co[6 * b + 3:6 * b + 6, :])
    # reflect guard columns for the horizontal gaussian
    for u in (u1, u2):
        nc.scalar.copy(u[:, :, 1:2], u[:, :, 3:4])
        nc.scalar.copy(u[:, :, 0:1], u[:, :, 4:5])
        nc.scalar.copy(u[:, :, WP - 2:WP - 1], u[:, :, WP - 4:WP - 3])
        nc.scalar.copy(u[:, :, WP - 1:WP], u[:, :, WP - 5:WP - 4])
    yield
    if gate():
        return

    # ---- horizontal gaussian (5-tap, Horner; final xC0 folded into Square) ----
    bl = []
    for u in (u1, u2):
        a1 = wt()
        a2 = wt()
        q1 = wt()
        b_ = wt()
        for hs in BS:
            nc.gpsimd.tensor_tensor(a1[:, hs, NW], u[:, hs, 1:1 + W],
                                    u[:, hs, 3:3 + W], AL.add)
            nc.gpsimd.tensor_tensor(a2[:, hs, NW], u[:, hs, 0:W],
                                    u[:, hs, 4:4 + W], AL.add)
            nc.vector.scalar_tensor_tensor(q1[:, hs, NW], a2[:, hs, NW], C2 / C1,
                                           a1[:, hs, NW], AL.mult, AL.add)
            nc.vector.scalar_tensor_tensor(b_[:, hs, INT], q1[:, hs, NW], C1 / C0,
                                           u[:, hs, INT], AL.mult, AL.add)
        # replicate guard columns for the 3-tap stage
        nc.scalar.copy(b_[:, :, 1:2], b_[:, :, 2:3])
        nc.scalar.copy(b_[:, :, WP - 2:WP - 1], b_[:, :, WP - 3:WP - 2])
        bl.append(b_)
    b1, b2 = bl
    yield
    if gate():
        return

    # ---- gradients (scaled by 1/C0), squares, s = gx^2+gy^2 ----
    gx = wt()
    ay = wt()
    gy = wt()
    sqx = wt()
    sqy = wt()
    s = wt()
    pxy = wt()
    md1 = pmask.tile([P, NT, W], U8, tag="m", name="m")
    ch = pmask.tile([P, NT, W], U8, tag="m", name="m")
    cv = pmask.tile([P, NT, W], U8, tag="m", name="m")
    for hs in BS:
        nc.gpsimd.tensor_tensor(gx[:, hs, NW], b1[:, hs, 3:3 + W],
                                b1[:, hs, 1:1 + W], AL.subtract)
        nc.gpsimd.tensor_tensor(ay[:, hs, NW], b2[:, hs, 1:1 + W],
                                b2[:, hs, 3:3 + W], AL.add)
        nc.vector.scalar_tensor_tensor(gy[:, hs, NW], b2[:, hs, INT], 2.0,
                                       ay[:, hs, NW], AL.mult, AL.add)
        nc.scalar.activation(sqx[:, hs, NW], gx[:, hs, NW], AF.Square, 0.0, C0)
        nc.scalar.activation(sqy[:, hs, NW], gy[:, hs, NW], AF.Square, 0.0, C0)
        nc.gpsimd.tensor_tensor(s[:, hs, INT], sqx[:, hs, NW], sqy[:, hs, NW],
                                AL.add)
        nc.gpsimd.tensor_tensor(pxy[:, hs, NW], gx[:, hs, NW], gy[:, hs, NW],
                                AL.mult)
        nc.vector.tensor_scalar(md1[:, hs, :], pxy[:, hs, NW], 0.0, None, AL.is_gt)
        nc.vector.scalar_tensor_tensor(ch[:, hs, :], sqx[:, hs, NW], float(T2),
                                       sqy[:, hs, NW], AL.mult, AL.is_ge)
        nc.vector.scalar_tensor_tensor(cv[:, hs, :], sqy[:, hs, NW], float(T2),
                                       sqx[:, hs, NW], AL.mult, AL.is_gt)
    nc.gpsimd.memset(s[:, :, 0:PAD], 0.0)
    nc.gpsimd.memset(s[:, :, WP - PAD:WP], 0.0)
    yield
    if gate():
        return

    # ---- row-shifted planes U[r]=s[r+1], D[r]=s[r-1] via exact PE permutation ----
    Upl = wt()
    Dpl = wt()
    for t in range(NT):
        for (ci, pl) in ((0, Upl), (1, Dpl)):
            psb = ppsum.tile([P, W], F32, tag="ps", name="ps")
            nc.tensor.matmul(psb[:], c_shm[:, ci, :], s[:, t, INT],
                             start=True, stop=True)
            nc.scalar.copy(pl[:, t, INT], psb[:])
    for pl in (Upl, Dpl):
        nc.gpsimd.memset(pl[:, :, 0:PAD], 0.0)
        nc.gpsimd.memset(pl[:, :, WP - PAD:WP], 0.0)
    for t in range(NT - 1):
        nc.sync.dma_start(Upl[127:128, t, INT], s[0:1, t + 1, INT])
    nc.sync.dma_start(Upl[127:128, NT - 1, INT], zeros[0:1, 0, :])
    for t in range(1, NT):
        nc.sync.dma_start(Dpl[0:1, t, INT], s[127:128, t - 1, INT])
    nc.gpsimd.memset(Dpl[0:1, 0, INT], 0.0)
    yield
    if gate():
        return

    # ---- neighbor maxes, axis selection, output ----
    mh = wt()
    mv = wt()
    md1m = wt()
    sel = wt()
    mag = wt()
    magc = wt()
    keep = wt()
    out_ = wt()
    for hs in BS:
        nc.vector.tensor_tensor(mh[:, hs, NW], s[:, hs, 1:1 + W],
                                s[:, hs, 3:3 + W], AL.max)
        nc.vector.tensor_tensor(mv[:, hs, NW], Upl[:, hs, INT], Dpl[:, hs, INT],
                                AL.max)
        nc.vector.tensor_tensor(md1m[:, hs, NW], Upl[:, hs, 3:3 + W],
                                Dpl[:, hs, 1:1 + W], AL.max)
        nc.vector.tensor_tensor(sel[:, hs, NW], Upl[:, hs, 1:1 + W],
                                Dpl[:, hs, 3:3 + W], AL.max)
        nc.vector.copy_predicated(sel[:, hs, NW], md1[:, hs, :], md1m[:, hs, NW])
        nc.vector.copy_predicated(sel[:, hs, NW], cv[:, hs, :], mv[:, hs, NW])
        nc.vector.copy_predicated(sel[:, hs, NW], ch[:, hs, :], mh[:, hs, NW])
        nc.scalar.activation(mag[:, hs, NW], s[:, hs, INT], AF.Sqrt, epsb[:], 1.0)
        nc.vector.tensor_scalar_min(magc[:, hs, NW], mag[:, hs, NW], 1.0)
        nc.vector.tensor_tensor(keep[:, hs, NW], s[:, hs, INT], sel[:, hs, NW],
                                AL.is_gt)
        nc.gpsimd.tensor_tensor(out_[:, hs, NW], magc[:, hs, NW],
                                keep[:, hs, NW], AL.mult)
        for c in range(3):
            nc.sync.dma_start(
                ydram[img, c].rearrange("(t p) w -> p t w", p=P)[:, hs, :],
                out_[:, hs, NW])
    yield
    if gate():
        return


def _build(nloop=1):
    import concourse.bacc as bacc
    import concourse.mybir as mybir
    from concourse import tile
    from contextlib import nullcontext
    F32 = mybir.dt.float32

    nc = bacc.Bacc("TRN2", target_bir_lowering=False, debug=False,
                   num_devices=NCORES)
    xdram = nc.declare_dram_parameter("xc", [NI, 3, H, W], F32, isOutput=False)
    c_vs_d = nc.declare_dram_parameter("vs", [P, NT, P], F32, isOutput=False)
    c_vd_d = nc.declare_dram_parameter("vd", [P, NT, P], F32, isOutput=False)
    c_vcor_d = nc.declare_dram_parameter("vcor", [36, 2, 18], F32, isOutput=False)
    c_shm_d = nc.declare_dram_parameter("shm", [P, 2, P], F32, isOutput=False)
    ydram = nc.declare_dram_parameter("y", [NI, 3, H, W], F32, isOutput=True)

    with tile.TileContext(nc) as tc:
        with tc.tile_pool(name="pconst", bufs=1) as pconst, \
             tc.tile_pool(name="pwork", bufs=21) as pwork, \
             tc.tile_pool(name="pmask", bufs=6) as pmask, \
             tc.tile_pool(name="psmall", bufs=2) as psmall, \
             tc.tile_pool(name="ppsum", bufs=6, space="PSUM") as ppsum:
            c_vs = pconst.tile([P, NT, P], F32, tag="cvs")
            nc.sync.dma_start(c_vs[:], c_vs_d[:])
            c_vd = pconst.tile([P, NT, P], F32, tag="cvd")
            nc.sync.dma_start(c_vd[:], c_vd_d[:])
            c_vcor = pconst.tile([36, 2, 18], F32, tag="cvcor")
            nc.sync.dma_start(c_vcor[:], c_vcor_d[:])
            c_shm = pconst.tile([P, 2, P], F32, tag="cshm")
            nc.sync.dma_start(c_shm[:], c_shm_d[:])
            zeros = pconst.tile([P, NT, W], F32, tag="zeros")
            nc.gpsimd.memset(zeros[:], 0.0)
            epsb = pconst.tile([P, 1], F32, tag="epsb")
            nc.gpsimd.memset(epsb[:], float(EPS))

            pools = (pwork, pmask, psmall, ppsum)
            tens = (xdram, ydram, c_vs, c_vd, c_vcor, c_shm, zeros, epsb)
            import os
            nrep = int(os.environ.get("KREPEAT", "1"))
            loop_cm = tc.For_i(0, nloop, 1) if nloop > 1 else nullcontext()
            with loop_cm:
                for rep in range(nrep):
                    gens = [_emit_image(nc, tc, pools, tens, img)
                            for img in range(NI)]
                    done = [False] * NI
                    while not all(done):
                        for i, gi in enumerate(gens):
                            if not done[i]:
                                try:
                                    next(gi)
                                except StopIteration:
                                    done[i] = True

    nc.compile()
    return nc


def _get_nc():
    if "nc" not in _CACHE:
        _CACHE["nc"] = _build()
        _CACHE["consts"] = _build_consts()
    return _CACHE["nc"], _CACHE["consts"]


def kernel(x):
    from concourse.bass_utils import run_bass_kernel_spmd
    x = np.ascontiguousarray(np.asarray(x), dtype=np.float32)
    assert x.shape == (16, 3, H, W), x.shape
    nc, consts = _get_nc()
    in_maps = []
    for c in range(NCORES):
        m = {"xc": x[NI * c:NI * (c + 1)]}
        m.update(consts)
        in_maps.append(m)
    res = run_bass_kernel_spmd(nc, in_maps, list(range(NCORES)))
    y = np.concatenate([res.results[c]["y"] for c in range(NCORES)], axis=0)
    return y.astype(np.float32)


if __name__ == "__main__":
    import golden
    rng = np.random.default_rng(0)
    x = rng.random((16, 3, H, W), dtype=np.float32)
    y = kernel(x)
    ref = golden.reference_np(x)
    d = y - ref
    print("L2 rel:", np.linalg.norm(d) / np.linalg.norm(ref))
    print("absmax:", np.abs(d).max(), " bigpix:", (np.abs(d) > 1e-3).sum())

